# revision 22
# baseline (speedup 1.0000x reference)
"""YOLO loss (nms_detection) Trainium2 Bass kernel.

Data parallel over 8 NeuronCores (4 images per core). Host prep casts and
packs inputs; all loss math runs on device:

  - dense stream per core: 6 fp16 channels per cell (conf, obj, xy, wh
    logits) packed [img, 128, 84 slots, 6]; class/label channels never
    touch the device densely (they only matter at obj cells).
  - obj rows (labels+preds+grid aux) are host-gathered into a small fp32
    side tensor; the sparse xy/wh/cls losses are computed on device from
    those rows, batched over the 3 layers.
  - IoU ignore mask: decode boxes on device; per (img, layer) the
    [128, S, M, 2] min/max/sub ops keep the (x,y) pair as the packed
    innermost dim so DVE runs them in 2x mode; relu on ACT; the
    intersection product on Pool; threshold test 3*inter >= a1+a2.
  - images are permuted so each slot position gets similar box counts
    across cores (per-slot M is the max over its 8 images).
  - activation table usage is phased (sigmoid set, then exp/ln set) so
    only 2 LoadActFuncSets are emitted.
"""

from contextlib import ExitStack

import numpy as np

ANCHORS = np.array([[116., 90.], [156., 198.], [373., 326.],
                    [30., 61.], [62., 45.], [59., 119.],
                    [10., 13.], [16., 30.], [33., 23.]], dtype=np.float32)
IMG_W = 416.0
P = 128
B_CORE = 4
N_CORES = 8
NCH = 6            # dense channels: conf, obj, px, py, pw, ph
STOT = 84          # dense slots: 4 (l0) + 16 (l1) + 64 (l2)
CH = 180           # sparse row channels
MAXB = 64          # reference top_k cap on boxes per image per layer

# per-layer: N cells (pos*anchor), slots S, grid W, slot offset
LAYERS = [
    dict(N=507,  S=4,  W=13.0, goff=0),
    dict(N=2028, S=16, W=26.0, goff=4),
    dict(N=8112, S=64, W=52.0, goff=20),
]

_NC_CACHE = {}


def _make_consts():
    # gc16 [128, 84, 6] fp16: (1/W, 1/W, gx/W, gy/W, aw/2W, ah/2W)
    # gcv  [128, 84] f32: valid mask
    gc = np.zeros((P, STOT, NCH), np.float32)
    gcv = np.zeros((P, STOT), np.float32)
    for li, lay in enumerate(LAYERS):
        W, N, S, goff = lay["W"], lay["N"], lay["S"], lay["goff"]
        c = np.arange(P * S)
        pos = c // 3
        gx = (pos % W).astype(np.float32)
        gy = (pos // W).astype(np.float32)
        aw = ANCHORS[3 * li + (c % 3), 0]
        ah = ANCHORS[3 * li + (c % 3), 1]
        valid = (c < N).astype(np.float32)
        # cell c -> slot goff + c//128, partition c%128
        s = goff + c // P
        p = c % P
        gc[p, s, 0] = 1.0 / W
        gc[p, s, 1] = 1.0 / W
        gc[p, s, 2] = np.where(valid, gx / W, 0.0)
        gc[p, s, 3] = np.where(valid, gy / W, 0.0)
        gc[p, s, 4] = np.where(valid, aw / (2.0 * W), 0.0)
        gc[p, s, 5] = np.where(valid, ah / (2.0 * W), 0.0)
        gcv[p, s] = valid
    return gc.astype(np.float16), gcv.astype(np.float16)


def _sel_mats(cap):
    # selection matrices for per-image sparse sums
    n_per = P // cap                     # images per sparse tile
    sels = []
    for h in range(B_CORE // n_per):     # one matrix per sparse tile
        m = np.zeros((P, B_CORE), np.float32)
        for g in range(n_per):
            img = h * n_per + g
            m[cap * g:cap * (g + 1), img] = 1.0
        sels.append(m)
    ones = np.ones((P, 1), np.float32)
    return sels, ones


def build_nc(Ms, cap):
    """Ms: [3][B_CORE] per-layer per-slot box counts. cap: 32 or 64."""
    import concourse.bass as bass
    import concourse.bacc as bacc
    import concourse.mybir as mybir
    from concourse.tile import TileContext

    F32 = mybir.dt.float32
    F16 = mybir.dt.float16
    ALU = mybir.AluOpType
    ACT = mybir.ActivationFunctionType
    AX = mybir.AxisListType

    n_per = P // cap                 # images per sparse tile
    n_sp = B_CORE // n_per           # number of sparse tiles
    btlen = sum(5 * Ms[l][j] for l in range(3) for j in range(B_CORE))
    btlen = max(btlen, 1)

    nc = bacc.Bacc()
    dn_d = nc.dram_tensor("dn", [B_CORE, P, STOT * NCH], F16,
                          kind="ExternalInput")
    sp_d = nc.dram_tensor("sp", [n_sp, P, 3 * CH], F32, kind="ExternalInput")
    bt_d = nc.dram_tensor("bt", [btlen], F16, kind="ExternalInput")
    gc_d = nc.dram_tensor("gc16", [P, STOT * NCH], F16, kind="ExternalInput")
    gv_d = nc.dram_tensor("gcv", [P, STOT], F16, kind="ExternalInput")
    se_d = nc.dram_tensor("sels", [P, n_sp * B_CORE + 1], F32,
                          kind="ExternalInput")
    loss_d = nc.dram_tensor("loss", [B_CORE, 1], F32, kind="ExternalOutput")

    def mkap(base, off_el, dims):
        return bass.AP(tensor=base.tensor, offset=base.offset + off_el,
                       ap=[base.ap[0]] + [list(d) for d in dims])

    with TileContext(nc) as tc, ExitStack() as ctx:
        cpool = ctx.enter_context(tc.tile_pool(name="consts", bufs=1))
        dpool = ctx.enter_context(tc.tile_pool(name="dense", bufs=1))
        ipool = ctx.enter_context(tc.tile_pool(name="iou", bufs=3))
        spool = ctx.enter_context(tc.tile_pool(name="sparse", bufs=1))
        pso = ctx.enter_context(
            tc.tile_pool(name="pso", bufs=1, space=bass.MemorySpace.PSUM))

        # ---- loads ----
        # HWDGE ring is serialized: order by first consumer. dn half 0
        # gates decode; bt gates the first IoU min; gc gates CXY.
        DN = dpool.tile([P, B_CORE, STOT, NCH], F16)
        dnf = DN[:]

        def dn_half(hf):
            nc.sync.dma_start(
                out=mkap(dnf, hf * 2 * STOT * NCH,
                         [[STOT * NCH, 2], [1, STOT * NCH]]),
                in_=bass.AP(tensor=dn_d[:].tensor,
                            offset=hf * 2 * P * STOT * NCH,
                            ap=[[STOT * NCH, P], [P * STOT * NCH, 2],
                                [1, STOT * NCH]]))

        dn_half(0)
        BT = cpool.tile([P, btlen], F16)
        nc.sync.dma_start(
            out=BT[:],
            in_=bass.AP(tensor=bt_d[:].tensor, offset=0,
                        ap=[[0, P], [1, btlen]]))
        GC = cpool.tile([P, STOT, NCH], F16)
        nc.sync.dma_start(out=GC[:], in_=gc_d[:])
        dn_half(1)
        SPT = [spool.tile([P, 3, CH], F32, name=f"spt{h}")
               for h in range(n_sp)]
        for h in range(n_sp):
            nc.sync.dma_start(
                out=mkap(SPT[h][:], 0, [[1, 3 * CH]]),
                in_=bass.AP(tensor=sp_d[:].tensor, offset=h * P * 3 * CH,
                            ap=[[3 * CH, P], [1, 3 * CH]]))
        GV = cpool.tile([P, STOT], F16)
        nc.sync.dma_start(out=GV[:], in_=gv_d[:])
        SEL = cpool.tile([P, n_sp * B_CORE + 1], F32)
        nc.sync.dma_start(out=SEL[:], in_=se_d[:])

        btf = BT[:]
        gcf = GC[:]

        def img4(off_el, dims):
            # const view broadcast over the 4-image dim
            return bass.AP(tensor=gcf.tensor, offset=gcf.offset + off_el,
                           ap=[gcf.ap[0], [0, B_CORE]] + [list(d) for d in dims])

        # ================= ACT phase 1: sigmoid set =================
        SXY = dpool.tile([P, B_CORE, STOT, 2], F16)
        C = dpool.tile([P, B_CORE, STOT], F16)
        for hf in range(2):
            i0, i1 = 2 * hf, 2 * hf + 2
            nc.scalar.activation(SXY[:, i0:i1], DN[:, i0:i1, :, 2:4],
                                 ACT.Sigmoid)
            nc.scalar.activation(C[:, i0:i1], DN[:, i0:i1, :, 0], ACT.Sigmoid)
        SPS = [spool.tile([P, 3, 2], F32, name=f"sps{h}") for h in range(n_sp)]
        SPCg = [spool.tile([P, 3, 80], F32, name=f"spc{h}")
                for h in range(n_sp)]
        for h in range(n_sp):
            nc.scalar.activation(SPS[h][:], SPT[h][:, :, 16:18], ACT.Sigmoid)
            nc.scalar.activation(SPCg[h][:], SPT[h][:, :, 100:180],
                                 ACT.Sigmoid)

        def img2(off_el, dims):
            return bass.AP(tensor=gcf.tensor, offset=gcf.offset + off_el,
                           ap=[gcf.ap[0], [0, 2]] + [list(d) for d in dims])

        # ================= dense decode (DVE) =================
        CXY = dpool.tile([P, B_CORE, STOT, 2], F16)
        for hf in range(2):
            i0, i1 = 2 * hf, 2 * hf + 2
            nc.vector.tensor_tensor(CXY[:, i0:i1], SXY[:, i0:i1],
                                    img2(0, [[NCH, STOT], [1, 2]]), ALU.mult)
            nc.vector.tensor_tensor(CXY[:, i0:i1], CXY[:, i0:i1],
                                    img2(2, [[NCH, STOT], [1, 2]]), ALU.add)

        # sparse logit chains (DVE) so all Exps can precede all Lns
        CXs, TXY, ECX, EPW, EC2 = {}, {}, {}, {}, {}
        LCX, LC2, TWH = {}, {}, {}
        for h in range(n_sp):
            CXs[h] = spool.tile([P, 3, 2], F32, name=f"cxs{h}")
            nc.vector.tensor_tensor(
                CXs[h][:], SPS[h][:],
                SPT[h][:, :, 9:10].broadcast_to([P, 3, 2]), ALU.mult)
            nc.vector.tensor_add(CXs[h][:], CXs[h][:], SPT[h][:, :, 7:9])
            TXY[h] = spool.tile([P, 3, 2], F32, name=f"txy{h}")
            nc.vector.tensor_tensor(
                TXY[h][:], SPT[h][:, :, 1:3],
                SPT[h][:, :, 10:11].broadcast_to([P, 3, 2]), ALU.mult)
            nc.vector.tensor_sub(TXY[h][:], TXY[h][:], SPT[h][:, :, 5:7])

        # ================= ACT phase 2: all Exp, then all Ln =================
        EWH = dpool.tile([P, B_CORE, STOT, 2], F16)
        E2 = dpool.tile([P, B_CORE, STOT], F32)
        for hf in range(2):
            i0, i1 = 2 * hf, 2 * hf + 2
            nc.scalar.activation(EWH[:, i0:i1], DN[:, i0:i1, :, 4:6], ACT.Exp)
            nc.scalar.activation(E2[:, i0:i1], C[:, i0:i1], ACT.Exp,
                                 scale=-1.0)
        for h in range(n_sp):
            ECX[h] = spool.tile([P, 3, 2], F32, name=f"ecx{h}")
            nc.scalar.activation(ECX[h][:], CXs[h][:], ACT.Exp, scale=-1.0)
            EPW[h] = spool.tile([P, 3, 2], F32, name=f"epw{h}")
            nc.scalar.activation(EPW[h][:], SPT[h][:, :, 18:20], ACT.Exp)
            EC2[h] = spool.tile([P, 3, 80], F32, name=f"ec2{h}")
            nc.scalar.activation(EC2[h][:], SPCg[h][:], ACT.Exp, scale=-1.0)
        L1 = dpool.tile([P, B_CORE, STOT], F16)
        nc.scalar.activation(L1[:], E2[:], ACT.Ln, bias=1.0)
        for h in range(n_sp):
            LCX[h] = spool.tile([P, 3, 2], F32, name=f"lcx{h}")
            nc.scalar.activation(LCX[h][:], ECX[h][:], ACT.Ln, bias=1.0)
            LC2[h] = spool.tile([P, 3, 80], F32, name=f"lc2{h}")
            nc.scalar.activation(LC2[h][:], EC2[h][:], ACT.Ln, bias=1.0)
            TWH[h] = spool.tile([P, 3, 2], F32, name=f"twh{h}")
            nc.scalar.activation(TWH[h][:], SPT[h][:, :, 3:5], ACT.Ln)

        HWT = dpool.tile([P, B_CORE, STOT, 2], F16)
        PM4 = dpool.tile([P, B_CORE, STOT, 4], F16)
        A13 = dpool.tile([P, B_CORE, STOT], F16)
        for hf in range(2):
            i0, i1 = 2 * hf, 2 * hf + 2
            nc.vector.tensor_tensor(HWT[:, i0:i1], EWH[:, i0:i1],
                                    img2(4, [[NCH, STOT], [1, 2]]), ALU.mult)
            nc.vector.tensor_add(PM4[:, i0:i1, :, 0:2], CXY[:, i0:i1],
                                 HWT[:, i0:i1])
            nc.vector.tensor_sub(PM4[:, i0:i1, :, 2:4], HWT[:, i0:i1],
                                 CXY[:, i0:i1])
            nc.vector.scalar_tensor_tensor(
                A13[:, i0:i1], HWT[:, i0:i1, :, 0], 4.0 / 3.0,
                HWT[:, i0:i1, :, 1], ALU.mult, ALU.mult)

        SMX = dpool.tile([P, B_CORE, STOT], F16)
        nc.vector.memset(SMX[:], -1.0e4)

        # ================= sparse losses =================
        SACC = spool.tile([P, n_sp, 3], F32)
        for h in range(n_sp):
            Sp = SPT[h][:]
            obj = SPT[h][:, :, 0:1]

            WH1 = spool.tile([P, 3], F32, name=f"wh1{h}")
            nc.vector.tensor_mul(WH1[:], SPT[h][:, :, 3], SPT[h][:, :, 4])
            SC = spool.tile([P, 3], F32, name=f"sc{h}")
            nc.vector.tensor_scalar(SC[:], WH1[:], -1.0, 2.0, ALU.mult,
                                    ALU.add)
            OSC = spool.tile([P, 3], F32, name=f"osc{h}")
            nc.vector.tensor_mul(OSC[:], SC[:], SPT[h][:, :, 0])
            oscv = OSC[:].broadcast_to([P, 3, 2])

            # xy bce on logits cx in (0,1)
            OMT = spool.tile([P, 3, 2], F32, name=f"omt{h}")
            nc.vector.tensor_scalar(OMT[:], TXY[h][:], -1.0, 1.0, ALU.mult,
                                    ALU.add)
            nc.vector.tensor_mul(OMT[:], OMT[:], CXs[h][:])
            nc.vector.tensor_add(OMT[:], OMT[:], LCX[h][:])
            SCR = spool.tile([P, 3, 2], F32, name=f"scr{h}")
            nc.vector.scalar_tensor_tensor(
                SCR[:], OMT[:], 1.0, oscv, ALU.mult, ALU.mult,
                accum_out=mkap(SACC[:], h * 3 + 0, [[1, 1]]))

            # wh squared error
            nc.vector.tensor_add(TWH[h][:], TWH[h][:], SPT[h][:, :, 13:15])
            nc.vector.tensor_mul(EPW[h][:], EPW[h][:], SPT[h][:, :, 11:13])
            nc.vector.tensor_sub(TWH[h][:], TWH[h][:], EPW[h][:])
            SQD = spool.tile([P, 3, 2], F32, name=f"sqd{h}")
            nc.scalar.activation(SQD[:], TWH[h][:], ACT.Square)
            SCR2 = spool.tile([P, 3, 2], F32, name=f"scr2{h}")
            nc.vector.scalar_tensor_tensor(
                SCR2[:], SQD[:], 0.5, oscv, ALU.mult, ALU.mult,
                accum_out=mkap(SACC[:], h * 3 + 1, [[1, 1]]))

            # cls bce on logits cls in (0,1)
            OM2 = spool.tile([P, 3, 80], F32, name=f"om2{h}")
            nc.vector.tensor_scalar(OM2[:], SPT[h][:, :, 20:100], -1.0, 1.0,
                                    ALU.mult, ALU.add)
            nc.vector.tensor_mul(OM2[:], OM2[:], SPCg[h][:])
            nc.vector.tensor_add(OM2[:], OM2[:], LC2[h][:])
            SCR3 = spool.tile([P, 3, 80], F32, name=f"scr3{h}")
            nc.vector.scalar_tensor_tensor(
                SCR3[:], OM2[:], 1.0, obj.broadcast_to([P, 3, 80]),
                ALU.mult, ALU.mult,
                accum_out=mkap(SACC[:], h * 3 + 2, [[1, 1]]))

        SSUM = spool.tile([P, n_sp], F32)
        nc.vector.tensor_reduce(SSUM[:], SACC[:], axis=AX.X, op=ALU.add)


        # ================= IoU per (img, layer) =================
        pm4f = PM4[:]
        bto = 0
        bt_off = {}
        for li in range(3):
            for j in range(B_CORE):
                m = Ms[li][j]
                bt_off[(li, j)] = bto
                bto += 5 * m
        chains = [(j, li) for j in range(B_CORE) for li in (2, 1, 0)
                  if Ms[li][j] > 0]
        WRXs, WDYs = {}, {}
        # wave 1: pure-DVE interval ops + ACT relu/copy compaction
        for j, li in chains:
            M = Ms[li][j]
            lay = LAYERS[li]
            S, goff = lay["S"], lay["goff"]
            o = bt_off[(li, j)]
            pm4v = mkap(pm4f, (j * STOT + goff) * 4,
                        [[4, S], [0, M], [1, 4]])
            btcv = mkap(btf, o, [[0, S], [4, M], [1, 4]])
            IJ = ipool.tile([P, S, M, 4], F16, tag=f"i{li}", name=f"i{li}_{j}")
            nc.vector.tensor_tensor(IJ[:], pm4v, btcv, ALU.min)
            ijf = IJ[:]
            WD = ipool.tile([P, S, M, 2], F16, tag=f"w{li}", name=f"w{li}_{j}")
            nc.vector.tensor_add(WD[:], mkap(ijf, 0, [[4 * M, S], [4, M], [1, 2]]),
                                 mkap(ijf, 2, [[4 * M, S], [4, M], [1, 2]]))
            wdf = WD[:]
            WRX = ipool.tile([P, S, M], F16, name=f"r{li}_{j}")
            nc.scalar.activation(WRX[:], mkap(wdf, 0, [[2 * M, S], [2, M]]),
                                 ACT.Relu)
            WDY = ipool.tile([P, S, M], F16, name=f"y{li}_{j}")
            nc.scalar.copy(WDY[:], mkap(wdf, 1, [[2 * M, S], [2, M]]))
            WRXs[(j, li)], WDYs[(j, li)] = WRX, WDY
        # wave 2: product, threshold, reduce (ACT results now ready)
        for j, li in chains:
            M = Ms[li][j]
            lay = LAYERS[li]
            S, goff = lay["S"], lay["goff"]
            o = bt_off[(li, j)]
            bt4v = mkap(btf, o + 4 * M, [[0, S], [1, M]])
            PRD = ipool.tile([P, S, M], F16, tag=f"p{li}", name=f"p{li}_{j}")
            nc.vector.tensor_mul(PRD[:], WRXs[(j, li)][:], WDYs[(j, li)][:])
            T = ipool.tile([P, S, M], F16, tag=f"t{li}", name=f"t{li}_{j}")
            nc.vector.tensor_tensor(T[:], PRD[:], bt4v, ALU.subtract)
            smxv = mkap(SMX[:], j * STOT + goff, [[1, S]])
            nc.vector.tensor_reduce(smxv, T[:], axis=AX.X, op=ALU.max)

        # ================= dense conf loss =================
        OBJ = dpool.tile([P, B_CORE, STOT], F16)
        CMP = dpool.tile([P, B_CORE, STOT], F16)
        WT = dpool.tile([P, B_CORE, STOT], F16)
        F = dpool.tile([P, B_CORE, STOT], F16)
        S2 = dpool.tile([P, B_CORE, STOT], F16)
        R1 = dpool.tile([P, B_CORE], F32)
        R2 = dpool.tile([P, B_CORE], F32)
        gvv2 = bass.AP(tensor=GV[:].tensor, offset=GV[:].offset,
                       ap=[GV[:].ap[0], [0, 2], [1, STOT]])
        for hf in range(2):
            i0, i1 = 2 * hf, 2 * hf + 2
            nc.vector.tensor_copy(OBJ[:, i0:i1], DN[:, i0:i1, :, 1])
            nc.vector.tensor_tensor(CMP[:, i0:i1], SMX[:, i0:i1],
                                    A13[:, i0:i1], ALU.is_lt)
            nc.vector.tensor_tensor(CMP[:, i0:i1], CMP[:, i0:i1], gvv2,
                                    ALU.mult)
            nc.vector.tensor_tensor(WT[:, i0:i1], CMP[:, i0:i1], OBJ[:, i0:i1],
                                    ALU.max)
            nc.vector.tensor_add(F[:, i0:i1], C[:, i0:i1], L1[:, i0:i1])
            nc.vector.tensor_mul(F[:, i0:i1], F[:, i0:i1], WT[:, i0:i1])
            nc.vector.tensor_reduce(R1[:, i0:i1], F[:, i0:i1], axis=AX.X,
                                    op=ALU.add)
            nc.vector.tensor_mul(S2[:, i0:i1], C[:, i0:i1], OBJ[:, i0:i1])
            nc.vector.tensor_reduce(R2[:, i0:i1], S2[:, i0:i1], axis=AX.X,
                                    op=ALU.add)
        FIN = dpool.tile([P, B_CORE], F32)
        nc.vector.tensor_sub(FIN[:], R1[:], R2[:])

        # ================= final combine =================
        PL = pso.tile([B_CORE, 1], F32)
        nc.tensor.matmul(PL[:], FIN[:], SEL[:, n_sp * B_CORE:], start=True,
                         stop=False)
        for h in range(n_sp):
            nc.tensor.matmul(PL[:], SEL[:, h * B_CORE:(h + 1) * B_CORE],
                             SSUM[:, h:h + 1], start=False, stop=(h == n_sp - 1))
        OUT = spool.tile([B_CORE, 1], F32)
        nc.scalar.copy(OUT[:], PL[:])
        nc.sync.dma_start(out=loss_d[:], in_=OUT[:])

    nc.finalize()
    return nc


def _plan(y_true):
    """Box counts -> image permutation + per-slot Ms + cap."""
    yt = np.asarray(y_true).reshape(32, 10647, 85)
    offs = [0, 507, 2535, 10647]
    counts = np.zeros((32, 3), np.int64)
    for li in range(3):
        counts[:, li] = (yt[:, offs[li]:offs[li + 1], 0] > 0.5).sum(1)
    order = np.argsort(counts[:, 2], kind="stable")
    perm = order.reshape(B_CORE, N_CORES)       # [slot, core] -> image
    cnt_cap = np.minimum(counts, MAXB)          # boxes capped like top_k
    Ms = [[int(cnt_cap[perm[j], li].max()) for j in range(B_CORE)]
          for li in range(3)]
    cap = 32 if counts.max() <= 32 else 64
    if counts.max() > MAXB:
        raise NotImplementedError("more than 64 true boxes per image/layer")
    return perm, Ms, cap, counts


def _prep_core_inputs(y_true, pred_13, pred_26, pred_52):
    yt = np.asarray(y_true).reshape(32, 10647, 85).astype(np.float32)
    ps = np.concatenate([np.asarray(p).reshape(32, -1, 85)
                         for p in (pred_13, pred_26, pred_52)],
                        axis=1).astype(np.float32)
    perm, Ms, cap, counts = _plan(y_true)
    n_per = P // cap
    n_sp = B_CORE // n_per

    offs = [0, 507, 2535]
    # dense packed stream [32, 128, 84, 6] fp16
    dn = np.zeros((32, P, STOT, NCH), np.float16)
    for li, lay in enumerate(LAYERS):
        N, S, goff = lay["N"], lay["S"], lay["goff"]
        c = np.arange(N)
        s = goff + c // P
        p = c % P
        cell = offs[li] + c
        dn[:, p, s, 0] = ps[:, cell, 0]
        dn[:, p, s, 1] = yt[:, cell, 0]
        dn[:, p, s, 2:6] = ps[:, cell, 1:5]

    # per-layer aux (grid / anchors) per flat cell
    aux = np.zeros((10647, 10), np.float32)
    for li, lay in enumerate(LAYERS):
        W, N = lay["W"], lay["N"]
        c = np.arange(N)
        pos = c // 3
        gx = (pos % W).astype(np.float32)
        gy = (pos // W).astype(np.float32)
        aw = ANCHORS[3 * li + (c % 3), 0]
        ah = ANCHORS[3 * li + (c % 3), 1]
        r = slice(offs[li], offs[li] + N)
        aux[r, 0] = gx
        aux[r, 1] = gy
        aux[r, 2] = gx / W
        aux[r, 3] = gy / W
        aux[r, 4] = 1.0 / W
        aux[r, 5] = W
        aux[r, 6] = aw / W
        aux[r, 7] = ah / W
        aux[r, 8] = np.log(IMG_W / aw)
        aux[r, 9] = np.log(IMG_W / ah)

    gc16, gcv = _make_consts()
    sels, ones = _sel_mats(cap)
    selcat = np.concatenate(sels + [ones], axis=1)

    in_maps = []
    for core in range(N_CORES):
        imgs = [int(perm[j, core]) for j in range(B_CORE)]
        dnc = np.ascontiguousarray(dn[imgs])

        sp = np.zeros((n_sp, P, 3, CH), np.float32)
        sp[:, :, :, 3:5] = 1.0     # pad yt wh -> ln() finite
        sp[:, :, :, 6 + 2] = 1.0   # pad invW etc: any finite nonzero
        sp[:, :, :, 11:13] = 1.0
        bt_parts = []
        for li in range(3):
            base = offs[li]
            N = LAYERS[li]["N"]
            for j in range(B_CORE):
                img = imgs[j]
                M = Ms[li][j]
                idx = np.nonzero(yt[img, base:base + N, 0] > 0.5)[0][:MAXB]
                k = len(idx)
                cells = base + idx
                if M > 0:
                    btc = np.full((M, 4), -100.0, np.float32)
                    bt4 = np.full((M,), 100.0, np.float32)
                    if k:
                        bxy = yt[img, cells, 1:3]
                        bwh = yt[img, cells, 3:5]
                        btc[:k, 0:2] = bxy + bwh * 0.5
                        btc[:k, 2:4] = -(bxy - bwh * 0.5)
                        bt4[:k] = bwh[:, 0] * bwh[:, 1] / 3.0
                    bt_parts += [btc.ravel(), bt4]
                if k:
                    h, g = j // n_per, j % n_per
                    rows = slice(cap * g, cap * g + k)
                    sp[h, rows, li, 0] = 1.0
                    sp[h, rows, li, 1:5] = yt[img, cells, 1:5]
                    sp[h, rows, li, 5:15] = aux[cells]
                    sp[h, rows, li, 16:20] = ps[img, cells, 1:5]
                    sp[h, rows, li, 20:100] = yt[img, cells, 5:85]
                    sp[h, rows, li, 100:180] = ps[img, cells, 5:85]
        bt = (np.concatenate(bt_parts).astype(np.float16) if bt_parts
              else np.zeros((1,), np.float16))
        m = {"dn": dnc.reshape(B_CORE, P, STOT * NCH),
             "sp": sp.reshape(n_sp, P, 3 * CH),
             "bt": bt,
             "gc16": gc16.reshape(P, STOT * NCH),
             "gcv": gcv,
             "sels": selcat}
        in_maps.append(m)
    return in_maps


def kernel(y_true, pred_13, pred_26, pred_52):
    from concourse.bass_utils import run_bass_kernel_spmd

    perm, Ms, cap, counts = _plan(y_true)
    key = (tuple(tuple(m) for m in Ms), cap)
    if key not in _NC_CACHE:
        _NC_CACHE[key] = build_nc(Ms, cap)
    nc = _NC_CACHE[key]

    in_maps = _prep_core_inputs(y_true, pred_13, pred_26, pred_52)
    res = run_bass_kernel_spmd(nc, in_maps, core_ids=list(range(N_CORES)))
    out = np.zeros((32,), np.float32)
    for core in range(N_CORES):
        vals = res.results[core]["loss"].reshape(B_CORE)
        for j in range(B_CORE):
            out[perm[j, core]] = vals[j]
    return out


# revision 23
# speedup vs baseline: 1.0018x; 1.0018x over previous
"""YOLO loss (nms_detection) Trainium2 Bass kernel.

Data parallel over 8 NeuronCores (4 images per core). Host prep casts and
packs inputs; all loss math runs on device:

  - dense stream per core: 6 fp16 channels per cell (conf, obj, xy, wh
    logits) packed [img, 128, 84 slots, 6]; class/label channels never
    touch the device densely (they only matter at obj cells).
  - obj rows (labels+preds+grid aux) are host-gathered into a small fp32
    side tensor; the sparse xy/wh/cls losses are computed on device from
    those rows, batched over the 3 layers.
  - IoU ignore mask: decode boxes on device; per (img, layer) the
    [128, S, M, 2] min/max/sub ops keep the (x,y) pair as the packed
    innermost dim so DVE runs them in 2x mode; relu on ACT; the
    intersection product on Pool; threshold test 3*inter >= a1+a2.
  - images are permuted so each slot position gets similar box counts
    across cores (per-slot M is the max over its 8 images).
  - activation table usage is phased (sigmoid set, then exp/ln set) so
    only 2 LoadActFuncSets are emitted.
"""

from contextlib import ExitStack

import numpy as np

ANCHORS = np.array([[116., 90.], [156., 198.], [373., 326.],
                    [30., 61.], [62., 45.], [59., 119.],
                    [10., 13.], [16., 30.], [33., 23.]], dtype=np.float32)
IMG_W = 416.0
P = 128
B_CORE = 4
N_CORES = 8
NCH = 6            # dense channels: conf, obj, px, py, pw, ph
STOT = 84          # dense slots: 4 (l0) + 16 (l1) + 64 (l2)
CH = 180           # sparse row channels
MAXB = 64          # reference top_k cap on boxes per image per layer

# per-layer: N cells (pos*anchor), slots S, grid W, slot offset
LAYERS = [
    dict(N=507,  S=4,  W=13.0, goff=0),
    dict(N=2028, S=16, W=26.0, goff=4),
    dict(N=8112, S=64, W=52.0, goff=20),
]

_NC_CACHE = {}


def _make_consts():
    # gc16 [128, 84, 6] fp16: (1/W, 1/W, gx/W, gy/W, aw/2W, ah/2W)
    # gcv  [128, 84] f32: valid mask
    gc = np.zeros((P, STOT, NCH), np.float32)
    gcv = np.zeros((P, STOT), np.float32)
    for li, lay in enumerate(LAYERS):
        W, N, S, goff = lay["W"], lay["N"], lay["S"], lay["goff"]
        c = np.arange(P * S)
        pos = c // 3
        gx = (pos % W).astype(np.float32)
        gy = (pos // W).astype(np.float32)
        aw = ANCHORS[3 * li + (c % 3), 0]
        ah = ANCHORS[3 * li + (c % 3), 1]
        valid = (c < N).astype(np.float32)
        # cell c -> slot goff + c//128, partition c%128
        s = goff + c // P
        p = c % P
        gc[p, s, 0] = 1.0 / W
        gc[p, s, 1] = 1.0 / W
        gc[p, s, 2] = np.where(valid, gx / W, 0.0)
        gc[p, s, 3] = np.where(valid, gy / W, 0.0)
        gc[p, s, 4] = np.where(valid, aw / (2.0 * W), 0.0)
        gc[p, s, 5] = np.where(valid, ah / (2.0 * W), 0.0)
        gcv[p, s] = valid
    return gc.astype(np.float16), gcv.astype(np.float16)


def _sel_mats(cap):
    # selection matrices for per-image sparse sums
    n_per = P // cap                     # images per sparse tile
    sels = []
    for h in range(B_CORE // n_per):     # one matrix per sparse tile
        m = np.zeros((P, B_CORE), np.float32)
        for g in range(n_per):
            img = h * n_per + g
            m[cap * g:cap * (g + 1), img] = 1.0
        sels.append(m)
    ones = np.ones((P, 1), np.float32)
    return sels, ones


def build_nc(Ms, cap):
    """Ms: [3][B_CORE] per-layer per-slot box counts. cap: 32 or 64."""
    import concourse.bass as bass
    import concourse.bacc as bacc
    import concourse.mybir as mybir
    from concourse.tile import TileContext

    F32 = mybir.dt.float32
    F16 = mybir.dt.float16
    ALU = mybir.AluOpType
    ACT = mybir.ActivationFunctionType
    AX = mybir.AxisListType

    n_per = P // cap                 # images per sparse tile
    n_sp = B_CORE // n_per           # number of sparse tiles
    btlen = sum(5 * Ms[l][j] for l in range(3) for j in range(B_CORE))
    btlen = max(btlen, 1)

    nc = bacc.Bacc()
    dn_d = nc.dram_tensor("dn", [B_CORE, P, STOT * NCH], F16,
                          kind="ExternalInput")
    sp_d = nc.dram_tensor("sp", [n_sp, P, 3 * CH], F32, kind="ExternalInput")
    bt_d = nc.dram_tensor("bt", [btlen], F16, kind="ExternalInput")
    gc_d = nc.dram_tensor("gc16", [P, STOT * NCH], F16, kind="ExternalInput")
    gv_d = nc.dram_tensor("gcv", [P, STOT], F16, kind="ExternalInput")
    se_d = nc.dram_tensor("sels", [P, n_sp * B_CORE + 1], F32,
                          kind="ExternalInput")
    loss_d = nc.dram_tensor("loss", [B_CORE, 1], F32, kind="ExternalOutput")

    def mkap(base, off_el, dims):
        return bass.AP(tensor=base.tensor, offset=base.offset + off_el,
                       ap=[base.ap[0]] + [list(d) for d in dims])

    with TileContext(nc) as tc, ExitStack() as ctx:
        cpool = ctx.enter_context(tc.tile_pool(name="consts", bufs=1))
        dpool = ctx.enter_context(tc.tile_pool(name="dense", bufs=1))
        ipool = ctx.enter_context(tc.tile_pool(name="iou", bufs=3))
        spool = ctx.enter_context(tc.tile_pool(name="sparse", bufs=1))
        pso = ctx.enter_context(
            tc.tile_pool(name="pso", bufs=1, space=bass.MemorySpace.PSUM))

        # ---- loads ----
        # HWDGE ring is serialized: order by first consumer. dn half 0
        # gates decode; bt gates the first IoU min; gc gates CXY.
        DN = dpool.tile([P, B_CORE, STOT, NCH], F16)
        dnf = DN[:]

        def dn_half(hf):
            nc.sync.dma_start(
                out=mkap(dnf, hf * 2 * STOT * NCH,
                         [[STOT * NCH, 2], [1, STOT * NCH]]),
                in_=bass.AP(tensor=dn_d[:].tensor,
                            offset=hf * 2 * P * STOT * NCH,
                            ap=[[STOT * NCH, P], [P * STOT * NCH, 2],
                                [1, STOT * NCH]]))

        dn_half(0)
        dn_half(1)
        GC = cpool.tile([P, STOT, NCH], F16)
        nc.sync.dma_start(out=GC[:], in_=gc_d[:])
        BT = cpool.tile([P, btlen], F16)
        nc.sync.dma_start(
            out=BT[:],
            in_=bass.AP(tensor=bt_d[:].tensor, offset=0,
                        ap=[[0, P], [1, btlen]]))
        SPT = [spool.tile([P, 3, CH], F32, name=f"spt{h}")
               for h in range(n_sp)]
        for h in range(n_sp):
            nc.sync.dma_start(
                out=mkap(SPT[h][:], 0, [[1, 3 * CH]]),
                in_=bass.AP(tensor=sp_d[:].tensor, offset=h * P * 3 * CH,
                            ap=[[3 * CH, P], [1, 3 * CH]]))
        GV = cpool.tile([P, STOT], F16)
        nc.sync.dma_start(out=GV[:], in_=gv_d[:])
        SEL = cpool.tile([P, n_sp * B_CORE + 1], F32)
        nc.sync.dma_start(out=SEL[:], in_=se_d[:])

        btf = BT[:]
        gcf = GC[:]

        def img4(off_el, dims):
            # const view broadcast over the 4-image dim
            return bass.AP(tensor=gcf.tensor, offset=gcf.offset + off_el,
                           ap=[gcf.ap[0], [0, B_CORE]] + [list(d) for d in dims])

        # ================= ACT phase 1: sigmoid set =================
        SXY = dpool.tile([P, B_CORE, STOT, 2], F16)
        C = dpool.tile([P, B_CORE, STOT], F16)
        for hf in range(2):
            i0, i1 = 2 * hf, 2 * hf + 2
            nc.scalar.activation(SXY[:, i0:i1], DN[:, i0:i1, :, 2:4],
                                 ACT.Sigmoid)
            nc.scalar.activation(C[:, i0:i1], DN[:, i0:i1, :, 0], ACT.Sigmoid)
        SPS = [spool.tile([P, 3, 2], F32, name=f"sps{h}") for h in range(n_sp)]
        SPCg = [spool.tile([P, 3, 80], F32, name=f"spc{h}")
                for h in range(n_sp)]
        for h in range(n_sp):
            nc.scalar.activation(SPS[h][:], SPT[h][:, :, 16:18], ACT.Sigmoid)
            nc.scalar.activation(SPCg[h][:], SPT[h][:, :, 100:180],
                                 ACT.Sigmoid)

        def img2(off_el, dims):
            return bass.AP(tensor=gcf.tensor, offset=gcf.offset + off_el,
                           ap=[gcf.ap[0], [0, 2]] + [list(d) for d in dims])

        # ================= dense decode (DVE) =================
        CXY = dpool.tile([P, B_CORE, STOT, 2], F16)
        for hf in range(2):
            i0, i1 = 2 * hf, 2 * hf + 2
            nc.vector.tensor_tensor(CXY[:, i0:i1], SXY[:, i0:i1],
                                    img2(0, [[NCH, STOT], [1, 2]]), ALU.mult)
            nc.vector.tensor_tensor(CXY[:, i0:i1], CXY[:, i0:i1],
                                    img2(2, [[NCH, STOT], [1, 2]]), ALU.add)

        # sparse logit chains (DVE) so all Exps can precede all Lns
        CXs, TXY, ECX, EPW, EC2 = {}, {}, {}, {}, {}
        LCX, LC2, TWH = {}, {}, {}
        for h in range(n_sp):
            CXs[h] = spool.tile([P, 3, 2], F32, name=f"cxs{h}")
            nc.vector.tensor_tensor(
                CXs[h][:], SPS[h][:],
                SPT[h][:, :, 9:10].broadcast_to([P, 3, 2]), ALU.mult)
            nc.vector.tensor_add(CXs[h][:], CXs[h][:], SPT[h][:, :, 7:9])
            TXY[h] = spool.tile([P, 3, 2], F32, name=f"txy{h}")
            nc.vector.tensor_tensor(
                TXY[h][:], SPT[h][:, :, 1:3],
                SPT[h][:, :, 10:11].broadcast_to([P, 3, 2]), ALU.mult)
            nc.vector.tensor_sub(TXY[h][:], TXY[h][:], SPT[h][:, :, 5:7])

        # ================= ACT phase 2: all Exp, then all Ln =================
        EWH = dpool.tile([P, B_CORE, STOT, 2], F16)
        E2 = dpool.tile([P, B_CORE, STOT], F32)
        for hf in range(2):
            i0, i1 = 2 * hf, 2 * hf + 2
            nc.scalar.activation(EWH[:, i0:i1], DN[:, i0:i1, :, 4:6], ACT.Exp)
            nc.scalar.activation(E2[:, i0:i1], C[:, i0:i1], ACT.Exp,
                                 scale=-1.0)
        for h in range(n_sp):
            ECX[h] = spool.tile([P, 3, 2], F32, name=f"ecx{h}")
            nc.scalar.activation(ECX[h][:], CXs[h][:], ACT.Exp, scale=-1.0)
            EPW[h] = spool.tile([P, 3, 2], F32, name=f"epw{h}")
            nc.scalar.activation(EPW[h][:], SPT[h][:, :, 18:20], ACT.Exp)
            EC2[h] = spool.tile([P, 3, 80], F32, name=f"ec2{h}")
            nc.scalar.activation(EC2[h][:], SPCg[h][:], ACT.Exp, scale=-1.0)
        L1 = dpool.tile([P, B_CORE, STOT], F16)
        nc.scalar.activation(L1[:], E2[:], ACT.Ln, bias=1.0)
        for h in range(n_sp):
            LCX[h] = spool.tile([P, 3, 2], F32, name=f"lcx{h}")
            nc.scalar.activation(LCX[h][:], ECX[h][:], ACT.Ln, bias=1.0)
            LC2[h] = spool.tile([P, 3, 80], F32, name=f"lc2{h}")
            nc.scalar.activation(LC2[h][:], EC2[h][:], ACT.Ln, bias=1.0)
            TWH[h] = spool.tile([P, 3, 2], F32, name=f"twh{h}")
            nc.scalar.activation(TWH[h][:], SPT[h][:, :, 3:5], ACT.Ln)

        HWT = dpool.tile([P, B_CORE, STOT, 2], F16)
        PM4 = dpool.tile([P, B_CORE, STOT, 4], F16)
        A13 = dpool.tile([P, B_CORE, STOT], F16)
        for hf in range(2):
            i0, i1 = 2 * hf, 2 * hf + 2
            nc.vector.tensor_tensor(HWT[:, i0:i1], EWH[:, i0:i1],
                                    img2(4, [[NCH, STOT], [1, 2]]), ALU.mult)
            nc.vector.tensor_add(PM4[:, i0:i1, :, 0:2], CXY[:, i0:i1],
                                 HWT[:, i0:i1])
            nc.vector.tensor_sub(PM4[:, i0:i1, :, 2:4], HWT[:, i0:i1],
                                 CXY[:, i0:i1])
            nc.vector.scalar_tensor_tensor(
                A13[:, i0:i1], HWT[:, i0:i1, :, 0], 4.0 / 3.0,
                HWT[:, i0:i1, :, 1], ALU.mult, ALU.mult)

        SMX = dpool.tile([P, B_CORE, STOT], F16)
        nc.vector.memset(SMX[:], -1.0e4)

        # ================= sparse losses =================
        SACC = spool.tile([P, n_sp, 3], F32)
        for h in range(n_sp):
            Sp = SPT[h][:]
            obj = SPT[h][:, :, 0:1]

            WH1 = spool.tile([P, 3], F32, name=f"wh1{h}")
            nc.vector.tensor_mul(WH1[:], SPT[h][:, :, 3], SPT[h][:, :, 4])
            SC = spool.tile([P, 3], F32, name=f"sc{h}")
            nc.vector.tensor_scalar(SC[:], WH1[:], -1.0, 2.0, ALU.mult,
                                    ALU.add)
            OSC = spool.tile([P, 3], F32, name=f"osc{h}")
            nc.vector.tensor_mul(OSC[:], SC[:], SPT[h][:, :, 0])
            oscv = OSC[:].broadcast_to([P, 3, 2])

            # xy bce on logits cx in (0,1)
            OMT = spool.tile([P, 3, 2], F32, name=f"omt{h}")
            nc.vector.tensor_scalar(OMT[:], TXY[h][:], -1.0, 1.0, ALU.mult,
                                    ALU.add)
            nc.vector.tensor_mul(OMT[:], OMT[:], CXs[h][:])
            nc.vector.tensor_add(OMT[:], OMT[:], LCX[h][:])
            SCR = spool.tile([P, 3, 2], F32, name=f"scr{h}")
            nc.vector.scalar_tensor_tensor(
                SCR[:], OMT[:], 1.0, oscv, ALU.mult, ALU.mult,
                accum_out=mkap(SACC[:], h * 3 + 0, [[1, 1]]))

            # wh squared error
            nc.vector.tensor_add(TWH[h][:], TWH[h][:], SPT[h][:, :, 13:15])
            nc.vector.tensor_mul(EPW[h][:], EPW[h][:], SPT[h][:, :, 11:13])
            nc.vector.tensor_sub(TWH[h][:], TWH[h][:], EPW[h][:])
            SQD = spool.tile([P, 3, 2], F32, name=f"sqd{h}")
            nc.scalar.activation(SQD[:], TWH[h][:], ACT.Square)
            SCR2 = spool.tile([P, 3, 2], F32, name=f"scr2{h}")
            nc.vector.scalar_tensor_tensor(
                SCR2[:], SQD[:], 0.5, oscv, ALU.mult, ALU.mult,
                accum_out=mkap(SACC[:], h * 3 + 1, [[1, 1]]))

            # cls bce on logits cls in (0,1)
            OM2 = spool.tile([P, 3, 80], F32, name=f"om2{h}")
            nc.vector.tensor_scalar(OM2[:], SPT[h][:, :, 20:100], -1.0, 1.0,
                                    ALU.mult, ALU.add)
            nc.vector.tensor_mul(OM2[:], OM2[:], SPCg[h][:])
            nc.vector.tensor_add(OM2[:], OM2[:], LC2[h][:])
            SCR3 = spool.tile([P, 3, 80], F32, name=f"scr3{h}")
            nc.vector.scalar_tensor_tensor(
                SCR3[:], OM2[:], 1.0, obj.broadcast_to([P, 3, 80]),
                ALU.mult, ALU.mult,
                accum_out=mkap(SACC[:], h * 3 + 2, [[1, 1]]))

        SSUM = spool.tile([P, n_sp], F32)
        nc.vector.tensor_reduce(SSUM[:], SACC[:], axis=AX.X, op=ALU.add)


        # ================= IoU per (img, layer) =================
        pm4f = PM4[:]
        bto = 0
        bt_off = {}
        for li in range(3):
            for j in range(B_CORE):
                m = Ms[li][j]
                bt_off[(li, j)] = bto
                bto += 5 * m
        chains = [(j, li) for j in range(B_CORE) for li in (2, 1, 0)
                  if Ms[li][j] > 0]
        WRXs, WDYs = {}, {}
        # wave 1: pure-DVE interval ops + ACT relu/copy compaction
        for j, li in chains:
            M = Ms[li][j]
            lay = LAYERS[li]
            S, goff = lay["S"], lay["goff"]
            o = bt_off[(li, j)]
            pm4v = mkap(pm4f, (j * STOT + goff) * 4,
                        [[4, S], [0, M], [1, 4]])
            btcv = mkap(btf, o, [[0, S], [4, M], [1, 4]])
            IJ = ipool.tile([P, S, M, 4], F16, tag=f"i{li}", name=f"i{li}_{j}")
            nc.vector.tensor_tensor(IJ[:], pm4v, btcv, ALU.min)
            ijf = IJ[:]
            WD = ipool.tile([P, S, M, 2], F16, tag=f"w{li}", name=f"w{li}_{j}")
            nc.vector.tensor_add(WD[:], mkap(ijf, 0, [[4 * M, S], [4, M], [1, 2]]),
                                 mkap(ijf, 2, [[4 * M, S], [4, M], [1, 2]]))
            wdf = WD[:]
            WRX = ipool.tile([P, S, M], F16, name=f"r{li}_{j}")
            nc.scalar.activation(WRX[:], mkap(wdf, 0, [[2 * M, S], [2, M]]),
                                 ACT.Relu)
            WDY = ipool.tile([P, S, M], F16, name=f"y{li}_{j}")
            nc.scalar.copy(WDY[:], mkap(wdf, 1, [[2 * M, S], [2, M]]))
            WRXs[(j, li)], WDYs[(j, li)] = WRX, WDY
        # wave 2: product, threshold, reduce (ACT results now ready)
        for j, li in chains:
            M = Ms[li][j]
            lay = LAYERS[li]
            S, goff = lay["S"], lay["goff"]
            o = bt_off[(li, j)]
            bt4v = mkap(btf, o + 4 * M, [[0, S], [1, M]])
            PRD = ipool.tile([P, S, M], F16, tag=f"p{li}", name=f"p{li}_{j}")
            nc.vector.tensor_mul(PRD[:], WRXs[(j, li)][:], WDYs[(j, li)][:])
            T = ipool.tile([P, S, M], F16, tag=f"t{li}", name=f"t{li}_{j}")
            nc.vector.tensor_tensor(T[:], PRD[:], bt4v, ALU.subtract)
            smxv = mkap(SMX[:], j * STOT + goff, [[1, S]])
            nc.vector.tensor_reduce(smxv, T[:], axis=AX.X, op=ALU.max)

        # ================= dense conf loss =================
        OBJ = dpool.tile([P, B_CORE, STOT], F16)
        CMP = dpool.tile([P, B_CORE, STOT], F16)
        WT = dpool.tile([P, B_CORE, STOT], F16)
        F = dpool.tile([P, B_CORE, STOT], F16)
        S2 = dpool.tile([P, B_CORE, STOT], F16)
        R1 = dpool.tile([P, B_CORE], F32)
        R2 = dpool.tile([P, B_CORE], F32)
        gvv2 = bass.AP(tensor=GV[:].tensor, offset=GV[:].offset,
                       ap=[GV[:].ap[0], [0, 2], [1, STOT]])
        for hf in range(2):
            i0, i1 = 2 * hf, 2 * hf + 2
            nc.vector.tensor_copy(OBJ[:, i0:i1], DN[:, i0:i1, :, 1])
            nc.vector.tensor_tensor(CMP[:, i0:i1], SMX[:, i0:i1],
                                    A13[:, i0:i1], ALU.is_lt)
            nc.vector.tensor_tensor(CMP[:, i0:i1], CMP[:, i0:i1], gvv2,
                                    ALU.mult)
            nc.vector.tensor_tensor(WT[:, i0:i1], CMP[:, i0:i1], OBJ[:, i0:i1],
                                    ALU.max)
            nc.vector.tensor_add(F[:, i0:i1], C[:, i0:i1], L1[:, i0:i1])
            nc.vector.tensor_mul(F[:, i0:i1], F[:, i0:i1], WT[:, i0:i1])
            nc.vector.tensor_reduce(R1[:, i0:i1], F[:, i0:i1], axis=AX.X,
                                    op=ALU.add)
            nc.vector.tensor_mul(S2[:, i0:i1], C[:, i0:i1], OBJ[:, i0:i1])
            nc.vector.tensor_reduce(R2[:, i0:i1], S2[:, i0:i1], axis=AX.X,
                                    op=ALU.add)
        FIN = dpool.tile([P, B_CORE], F32)
        nc.vector.tensor_sub(FIN[:], R1[:], R2[:])

        # ================= final combine =================
        PL = pso.tile([B_CORE, 1], F32)
        nc.tensor.matmul(PL[:], FIN[:], SEL[:, n_sp * B_CORE:], start=True,
                         stop=False)
        for h in range(n_sp):
            nc.tensor.matmul(PL[:], SEL[:, h * B_CORE:(h + 1) * B_CORE],
                             SSUM[:, h:h + 1], start=False, stop=(h == n_sp - 1))
        OUT = spool.tile([B_CORE, 1], F32)
        nc.scalar.copy(OUT[:], PL[:])
        nc.sync.dma_start(out=loss_d[:], in_=OUT[:])

    nc.finalize()
    return nc


def _plan(y_true):
    """Box counts -> image permutation + per-slot Ms + cap."""
    yt = np.asarray(y_true).reshape(32, 10647, 85)
    offs = [0, 507, 2535, 10647]
    counts = np.zeros((32, 3), np.int64)
    for li in range(3):
        counts[:, li] = (yt[:, offs[li]:offs[li + 1], 0] > 0.5).sum(1)
    order = np.argsort(counts[:, 2], kind="stable")
    perm = order.reshape(B_CORE, N_CORES)       # [slot, core] -> image
    cnt_cap = np.minimum(counts, MAXB)          # boxes capped like top_k
    Ms = [[int(cnt_cap[perm[j], li].max()) for j in range(B_CORE)]
          for li in range(3)]
    cap = 32 if counts.max() <= 32 else 64
    if counts.max() > MAXB:
        raise NotImplementedError("more than 64 true boxes per image/layer")
    return perm, Ms, cap, counts


def _prep_core_inputs(y_true, pred_13, pred_26, pred_52):
    yt = np.asarray(y_true).reshape(32, 10647, 85).astype(np.float32)
    ps = np.concatenate([np.asarray(p).reshape(32, -1, 85)
                         for p in (pred_13, pred_26, pred_52)],
                        axis=1).astype(np.float32)
    perm, Ms, cap, counts = _plan(y_true)
    n_per = P // cap
    n_sp = B_CORE // n_per

    offs = [0, 507, 2535]
    # dense packed stream [32, 128, 84, 6] fp16
    dn = np.zeros((32, P, STOT, NCH), np.float16)
    for li, lay in enumerate(LAYERS):
        N, S, goff = lay["N"], lay["S"], lay["goff"]
        c = np.arange(N)
        s = goff + c // P
        p = c % P
        cell = offs[li] + c
        dn[:, p, s, 0] = ps[:, cell, 0]
        dn[:, p, s, 1] = yt[:, cell, 0]
        dn[:, p, s, 2:6] = ps[:, cell, 1:5]

    # per-layer aux (grid / anchors) per flat cell
    aux = np.zeros((10647, 10), np.float32)
    for li, lay in enumerate(LAYERS):
        W, N = lay["W"], lay["N"]
        c = np.arange(N)
        pos = c // 3
        gx = (pos % W).astype(np.float32)
        gy = (pos // W).astype(np.float32)
        aw = ANCHORS[3 * li + (c % 3), 0]
        ah = ANCHORS[3 * li + (c % 3), 1]
        r = slice(offs[li], offs[li] + N)
        aux[r, 0] = gx
        aux[r, 1] = gy
        aux[r, 2] = gx / W
        aux[r, 3] = gy / W
        aux[r, 4] = 1.0 / W
        aux[r, 5] = W
        aux[r, 6] = aw / W
        aux[r, 7] = ah / W
        aux[r, 8] = np.log(IMG_W / aw)
        aux[r, 9] = np.log(IMG_W / ah)

    gc16, gcv = _make_consts()
    sels, ones = _sel_mats(cap)
    selcat = np.concatenate(sels + [ones], axis=1)

    in_maps = []
    for core in range(N_CORES):
        imgs = [int(perm[j, core]) for j in range(B_CORE)]
        dnc = np.ascontiguousarray(dn[imgs])

        sp = np.zeros((n_sp, P, 3, CH), np.float32)
        sp[:, :, :, 3:5] = 1.0     # pad yt wh -> ln() finite
        sp[:, :, :, 6 + 2] = 1.0   # pad invW etc: any finite nonzero
        sp[:, :, :, 11:13] = 1.0
        bt_parts = []
        for li in range(3):
            base = offs[li]
            N = LAYERS[li]["N"]
            for j in range(B_CORE):
                img = imgs[j]
                M = Ms[li][j]
                idx = np.nonzero(yt[img, base:base + N, 0] > 0.5)[0][:MAXB]
                k = len(idx)
                cells = base + idx
                if M > 0:
                    btc = np.full((M, 4), -100.0, np.float32)
                    bt4 = np.full((M,), 100.0, np.float32)
                    if k:
                        bxy = yt[img, cells, 1:3]
                        bwh = yt[img, cells, 3:5]
                        btc[:k, 0:2] = bxy + bwh * 0.5
                        btc[:k, 2:4] = -(bxy - bwh * 0.5)
                        bt4[:k] = bwh[:, 0] * bwh[:, 1] / 3.0
                    bt_parts += [btc.ravel(), bt4]
                if k:
                    h, g = j // n_per, j % n_per
                    rows = slice(cap * g, cap * g + k)
                    sp[h, rows, li, 0] = 1.0
                    sp[h, rows, li, 1:5] = yt[img, cells, 1:5]
                    sp[h, rows, li, 5:15] = aux[cells]
                    sp[h, rows, li, 16:20] = ps[img, cells, 1:5]
                    sp[h, rows, li, 20:100] = yt[img, cells, 5:85]
                    sp[h, rows, li, 100:180] = ps[img, cells, 5:85]
        bt = (np.concatenate(bt_parts).astype(np.float16) if bt_parts
              else np.zeros((1,), np.float16))
        m = {"dn": dnc.reshape(B_CORE, P, STOT * NCH),
             "sp": sp.reshape(n_sp, P, 3 * CH),
             "bt": bt,
             "gc16": gc16.reshape(P, STOT * NCH),
             "gcv": gcv,
             "sels": selcat}
        in_maps.append(m)
    return in_maps


def kernel(y_true, pred_13, pred_26, pred_52):
    from concourse.bass_utils import run_bass_kernel_spmd

    perm, Ms, cap, counts = _plan(y_true)
    key = (tuple(tuple(m) for m in Ms), cap)
    if key not in _NC_CACHE:
        _NC_CACHE[key] = build_nc(Ms, cap)
    nc = _NC_CACHE[key]

    in_maps = _prep_core_inputs(y_true, pred_13, pred_26, pred_52)
    res = run_bass_kernel_spmd(nc, in_maps, core_ids=list(range(N_CORES)))
    out = np.zeros((32,), np.float32)
    for core in range(N_CORES):
        vals = res.results[core]["loss"].reshape(B_CORE)
        for j in range(B_CORE):
            out[perm[j, core]] = vals[j]
    return out
